# revision 16
# baseline (speedup 1.0000x reference)
"""Trainium2 Bass kernel for nn_CausalLayer (bilinear causal mixing layer).

Math (per batch b):
    E = ae[x]                                # [L, D] gather
    S[i,j] = E_i @ w @ E_j                   # bilinear pairwise score
    coef[i,j] = (i+1)/(j+1) for i<j else 0
    res[:,j] = bx[:,j] + sum_i coef[i,j]*S[i,j]*bx[:,i]

Chunked linear-attention identity: with a_i = w^T E_i and y_i = (i+1)*bx_i,
    res_j = bx_j + (1/(j+1)) * [ M_cj @ E_j + sum_{i<j, same chunk} (a_i.E_j) y_i ]
    M_c   = sum_{i in chunks < c} y_i a_i^T      (rank-D running state, [D, H])

The device computes only the correction (bf16); the host adds bx in f32.

fp8 DoubleRow: the two BX-consuming matmuls (state update Ap^T@BX and the
masked intra block St^T@BX) run in fp8e4m3 DoubleRow mode (0.5 cyc/row, 2x PE
throughput). bx is loaded fp8-only in the DoubleRow-interleaved layout
(pairs of tokens p / 64+p share a partition), halving bx DMA. A 1/4096 scale
is folded into the St/Ap coefficient tables so all fp8 values stay well under
the e4m3 max of 448; the inverse 4096 factor rides the final output scale.
The state matrix M and its apply (Et^T @ M) stay bf16 for precision.

The chunk loop is software-pipelined with a one-chunk skew: iteration t
produces chunk t's small operands, and consumes chunk t-1's H-wide matmuls,
so the PE never waits on the vector/scalar engines. Loads ride the SP HWDGE
ring, stores the ACT ring.

Sharding: batch-parallel, 2 of 16 batches per core; tables replicated.
"""

import os
import sys

for _p in ("/opt/trn_rl_repo", "/root/.axon_site/_ro/trn_rl_repo"):
    if os.path.isdir(_p) and _p not in sys.path:
        sys.path.insert(0, _p)

import numpy as np

B, L, H = 16, 2048, 768
V, D = 30000, 64
NCORES = 8
BPC = B // NCORES
C = 128
NCH = L // C
ROWS = BPC * L
IOCH = 4
NG = BPC * NCH
FS = 4096.0                # fp8 range-folding scale

_compiled = {}


def _np_consts():
    import ml_dtypes

    i = np.arange(C, dtype=np.float64)
    cmask = np.zeros((C, NCH * C), np.float32)
    consts = np.zeros((C, 2 * NCH), np.float32)
    for c in range(NCH):
        gi = c * C + i
        cmask[:, c * C:(c + 1) * C] = np.where(
            i[:, None] < i[None, :], (gi + 1.0)[:, None] / FS, 0.0
        ).astype(np.float32)
        consts[:, c] = ((gi + 1.0) / FS).astype(np.float32)
        consts[:, NCH + c] = (FS / (gi + 1.0)).astype(np.float32)
    return cmask.astype(ml_dtypes.bfloat16), consts


def _build():
    key = "v4"
    if key in _compiled:
        return _compiled[key]

    import concourse.bacc as bacc
    import concourse.bass as bass
    import concourse.mybir as mybir
    import concourse.tile as tile
    from concourse.masks import make_identity

    f32 = mybir.dt.float32
    i32 = mybir.dt.int32
    bf16 = mybir.dt.bfloat16
    fp8 = mybir.dt.float8e4
    Copy = mybir.ActivationFunctionType.Copy
    DR = mybir.MatmulPerfMode.DoubleRow

    nc = bacc.Bacc(
        "TRN2",
        target_bir_lowering=False,
        debug=False,
        enable_asserts=False,
        num_devices=NCORES,
    )

    # bx8: fp8, DoubleRow layout: row (g*64+p) = [bx[g*128+p] | bx[g*128+64+p]]
    bx_d = nc.dram_tensor("bx8", [NG * 64, 2 * H], fp8, kind="ExternalInput").ap()
    idx_d = nc.dram_tensor("idx", [C, BPC * NCH], i32, kind="ExternalInput").ap()
    eaw_d = nc.dram_tensor("eaw", [V, 2 * D], bf16, kind="ExternalInput").ap()
    cm_d = nc.dram_tensor("cmask", [C, NCH * C], bf16, kind="ExternalInput").ap()
    ct_d = nc.dram_tensor("consts", [C, 2 * NCH], f32, kind="ExternalInput").ap()
    out_d = nc.dram_tensor("out", [ROWS, H], bf16, kind="ExternalOutput").ap()

    mult = mybir.AluOpType.mult
    K2 = IOCH // 2

    with tile.TileContext(nc) as tc:
        with (
            tc.tile_pool(name="const", bufs=1) as cpool,
            tc.tile_pool(name="bxp", bufs=3) as bxpool,
            tc.tile_pool(name="outp", bufs=2) as outpool,
            tc.tile_pool(name="sm", bufs=4) as smpool,
            tc.tile_pool(name="eap", bufs=8) as eapool,
            tc.tile_pool(name="mp", bufs=2) as mpool,
            tc.tile_pool(name="ps_small", bufs=1, space="PSUM") as ps_small,
            tc.tile_pool(name="ps_out", bufs=2, space="PSUM") as ps_out,
            tc.tile_pool(name="ps_m", bufs=1, space="PSUM") as ps_m,
        ):
            ident16 = cpool.tile([C, C], bf16)
            make_identity(nc, ident16[:])
            idx_s = cpool.tile([C, BPC * NCH], i32)
            nc.sync.dma_start(out=idx_s[:], in_=idx_d[:, :])
            consts_s = cpool.tile([C, 2 * NCH], f32)
            nc.sync.dma_start(out=consts_s[:], in_=ct_d[:, :])
            cmask_s = cpool.tile([C, NCH * C], bf16)
            nc.scalar.dma_start(out=cmask_s[:], in_=cm_d[:, :])

            M_p = None
            prev = None
            Ms_pending = None

            for t in range(NG + 1):
                made = None
                if t < NG:
                    b, c = divmod(t, NCH)
                    if c == 0:
                        M_p = ps_m.tile([D, H], f32, name=f"M_p_b{b}", tag="M_p")
                    if t % IOCH == 0:
                        # [IOCH*64, 2H] dram rows -> [128, K2*2H] tile:
                        # partition (two*64+p) col-block k2 = chunk 2*k2+two
                        BXW = bxpool.tile([C, K2 * 2 * H], fp8, name="BXW", tag="BXW")
                        nc.sync.dma_start(
                            out=BXW[:].rearrange(
                                "(two p) (k h) -> p two k h", two=2, k=K2
                            ),
                            in_=bx_d[t * 64:(t + IOCH) * 64, :].rearrange(
                                "(k two p) h -> p two k h", two=2, p=64
                            ),
                        )
                    co = t % IOCH
                    beta = (co % 2) * 64
                    # [64, 2, H] DoubleRow rhs view for this chunk
                    BX = BXW[beta:beta + 64, (co // 2) * 2 * H:(co // 2 + 1) * 2 * H]

                    EA = eapool.tile([C, 2 * D], bf16, name="EA", tag="EA")
                    nc.gpsimd.indirect_dma_start(
                        out=EA[:],
                        out_offset=None,
                        in_=eaw_d[:, :],
                        in_offset=bass.IndirectOffsetOnAxis(
                            ap=idx_s[:, t:t + 1], axis=0
                        ),
                    )

                    ea_p = ps_small.tile([D, 2 * C], bf16, name="ea_p", tag="ea_p")
                    nc.tensor.transpose(
                        out=ea_p[:, 0:C], in_=EA[:, 0:D], identity=ident16[:]
                    )
                    nc.tensor.transpose(
                        out=ea_p[:, C:2 * C], in_=EA[:, D:2 * D], identity=ident16[:]
                    )
                    Et = smpool.tile([D, C], bf16, name="Et", tag="Et")
                    nc.scalar.activation(out=Et[:], in_=ea_p[:, 0:C], func=Copy)
                    At = smpool.tile([D, C], bf16, name="At", tag="At")
                    nc.vector.tensor_scalar_mul(
                        out=At[:], in0=ea_p[:, C:2 * C], scalar1=1.0
                    )

                    # Ap_dr[p, t2*D+d] = A[t2*64+p, d]*(i+1)/FS at base beta
                    Ap = smpool.tile([C, 2 * D], fp8, name="Ap", tag="Ap")
                    nc.vector.tensor_scalar_mul(
                        out=Ap[beta:beta + 64, 0:D],
                        in0=EA[0:64, D:2 * D],
                        scalar1=consts_s[0:64, c:c + 1],
                    )
                    nc.vector.tensor_scalar_mul(
                        out=Ap[beta:beta + 64, D:2 * D],
                        in0=EA[64:C, D:2 * D],
                        scalar1=consts_s[64:C, c:c + 1],
                    )

                    s_p = ps_small.tile([C, C], f32, name="s_p", tag="s_p")
                    nc.tensor.matmul(
                        out=s_p[:], lhsT=At[:], rhs=Et[:], start=True, stop=True,
                    )
                    # St_dr[p, t2*C+j] = (S*cmask)[t2*64+p, j] at base beta
                    St = smpool.tile([C, 2 * C], fp8, name="St", tag="St")
                    nc.vector.tensor_tensor(
                        out=St[beta:beta + 64, 0:C],
                        in0=s_p[0:64, :],
                        in1=cmask_s[0:64, c * C:(c + 1) * C],
                        op=mult,
                    )
                    nc.vector.tensor_tensor(
                        out=St[beta:beta + 64, C:2 * C],
                        in0=s_p[64:C, :],
                        in1=cmask_s[64:C, c * C:(c + 1) * C],
                        op=mult,
                    )
                    made = (c, beta, BX, St, Et, Ap)

                if prev is not None:
                    c, beta, BXc, Stc, Etc, Apc = prev
                    g = t - 1

                    M_s = Ms_pending if c > 0 else None
                    bxv = BXc.rearrange("p (two h) -> p two h", two=2)

                    if c < NCH - 1:
                        for lo, hi in ((0, 512), (512, H)):
                            nc.tensor.matmul(
                                out=M_p[:, lo:hi],
                                lhsT=Apc[beta:beta + 64, :].rearrange(
                                    "p (two d) -> p two d", two=2
                                ),
                                rhs=bxv[:, :, lo:hi],
                                start=(c == 0),
                                stop=True,
                                perf_mode=DR,
                                skip_group_check=True,
                            )
                        Ms_pending = mpool.tile([D, H], bf16, name="M_s", tag="M_s")
                        nc.vector.tensor_scalar_mul(
                            out=Ms_pending[:], in0=M_p[:], scalar1=1.0
                        )

                    out_p = ps_out.tile([C, H], f32, name="out_p", tag="out_p")
                    for lo, hi in ((0, 512), (512, H)):
                        nc.tensor.matmul(
                            out=out_p[:, lo:hi],
                            lhsT=Stc[beta:beta + 64, :].rearrange(
                                "p (two j) -> p two j", two=2
                            ),
                            rhs=bxv[:, :, lo:hi],
                            start=True,
                            stop=(c == 0),
                            perf_mode=DR,
                        )
                    if c > 0:
                        for lo, hi in ((0, 512), (512, H)):
                            nc.tensor.matmul(
                                out=out_p[:, lo:hi],
                                lhsT=Etc[:],
                                rhs=M_s[:, lo:hi],
                                start=False,
                                stop=True,
                            )

                    if g % IOCH == 0:
                        OUTW = outpool.tile(
                            [C, IOCH * H], bf16, name="OUTW", tag="OUTW"
                        )
                    out_s = OUTW[:, (g % IOCH) * H:(g % IOCH + 1) * H]
                    nc.scalar.activation(
                        out=out_s,
                        in_=out_p[:],
                        func=Copy,
                        scale=consts_s[:, NCH + c:NCH + c + 1],
                    )
                    if g % IOCH == IOCH - 1:
                        nc.scalar.dma_start(
                            out=out_d[(g - IOCH + 1) * C:(g + 1) * C, :].rearrange(
                                "(k p) h -> p k h", k=IOCH
                            ),
                            in_=OUTW[:].rearrange("p (k h) -> p k h", k=IOCH),
                        )

                prev = made

    import concourse.mybir as mybir

    for blk in nc.m.functions[0].blocks:
        last = None
        for inst in blk.instructions:
            if getattr(inst, "engine", None) != mybir.EngineType.PE:
                continue
            if not isinstance(inst, mybir.InstMatmult):
                if isinstance(inst, (mybir.InstLdweights,)):
                    last = None
                continue
            if (
                last is not None
                and not inst.is_transpose
                and not last.is_transpose
                and inst.ins[1].memref == last.ins[1].memref
                and inst.ins[1].offset == last.ins[1].offset
                and inst.ins[1].ap == last.ins[1].ap
            ):
                inst.ldweights = True
            last = inst

    nc.compile()
    _compiled[key] = nc
    return nc


def _in_maps(bert_x, x, ae, w):
    import ml_dtypes

    bx = np.asarray(bert_x, dtype=np.float32)
    x = np.asarray(x)
    ae = np.asarray(ae, dtype=np.float32)
    w = np.asarray(w, dtype=np.float32)
    eaw = np.ascontiguousarray(
        np.concatenate([ae, ae @ w], axis=1).astype(ml_dtypes.bfloat16)
    )
    cmask, consts = _np_consts()
    xr = x.reshape(B, NCH, C).transpose(0, 2, 1).astype(np.int32)
    # DoubleRow bx8 layout: [NG*64, 2H], row (g*64+p) = [bx_g[p] | bx_g[64+p]]
    bx8_all = (
        bx.reshape(B * NCH, 2, 64, H)
        .transpose(0, 2, 1, 3)
        .reshape(B * NCH * 64, 2 * H)
        .astype(ml_dtypes.float8_e4m3fn)
    )
    maps = []
    for k in range(NCORES):
        maps.append(
            {
                "bx8": np.ascontiguousarray(
                    bx8_all[k * NG * 64:(k + 1) * NG * 64]
                ),
                "idx": np.ascontiguousarray(
                    np.concatenate([xr[k * BPC + b] for b in range(BPC)], axis=1)
                ),
                "eaw": eaw,
                "cmask": cmask,
                "consts": consts,
            }
        )
    return maps


def _run(bert_x, x, ae, w, trace=False):
    from concourse import bass_utils

    nc = _build()
    bert_x = np.asarray(bert_x, dtype=np.float32)
    maps = _in_maps(bert_x, x, ae, w)
    res = bass_utils.run_bass_kernel_spmd(
        nc, maps, core_ids=list(range(NCORES)), trace=trace
    )
    corr = np.concatenate(
        [
            np.asarray(res.results[k]["out"])
            .astype(np.float32)
            .reshape(BPC, L, H)
            for k in range(NCORES)
        ],
        axis=0,
    )
    out = bert_x + corr
    return out, res


def kernel(bert_x, x, ae, w):
    out, _ = _run(bert_x, x, ae, w, trace=False)
    return out


# revision 20
# speedup vs baseline: 1.2663x; 1.2663x over previous
"""Trainium2 Bass kernel for nn_CausalLayer (bilinear causal mixing layer).

Math (per batch b):
    E = ae[x]                                # [L, D] gather
    S[i,j] = E_i @ w @ E_j                   # bilinear pairwise score
    coef[i,j] = (i+1)/(j+1) for i<j else 0
    res[:,j] = bx[:,j] + sum_i coef[i,j]*S[i,j]*bx[:,i]

Rather than materializing the [L, L] score matrix (O(L^2 H) flops), we use the
chunked linear-attention identity. With a_i = w^T E_i and y_i = (i+1)*bx_i:

    res_j = bx_j + (1/(j+1)) * [ M_cj @ E_j + sum_{i<j, same chunk} (a_i.E_j) y_i ]
    M_c   = sum_{i in chunks < c} y_i a_i^T      (rank-D running state, [D, H])

Per 128-token chunk that is: a few tiny [*,64/128] matmuls, one masked [128,128]
score block, and three [*,768] matmuls -- O(L*C*(D+H) + L*D*H) total, 16x fewer
flops than the reference einsum, which puts the kernel at the HBM roofline
(bf16 bx in + f32 res out + gathers ~= 21 MB/core).

Sharding: batch-parallel, 2 of 16 batches per NeuronCore across 8 cores; ae/w
and the small constant tables are replicated. No cross-core communication.
"""

import os
import sys

for _p in ("/opt/trn_rl_repo", "/root/.axon_site/_ro/trn_rl_repo"):
    if os.path.isdir(_p) and _p not in sys.path:
        sys.path.insert(0, _p)

import numpy as np

B, L, H = 16, 2048, 768
V, D = 30000, 64
NCORES = 8
BPC = B // NCORES          # batches per core
C = 128                    # chunk (tile) size along sequence
NCH = L // C               # chunks per batch
ROWS = BPC * L             # bx rows per core

# dtype for the matmul path. This build is tuned for "bf16" (the fused gather
# table and transposes are bf16); measured scale-relative absmax error vs the
# fp32 reference is ~3.3e-3 with fp32 PSUM accumulation throughout.
BIG_DT = "bf16"

_compiled = {}


def _np_consts():
    i = np.arange(C, dtype=np.float64)
    cmask = np.zeros((C, NCH * C), np.float32)
    consts = np.zeros((C, 2 * NCH), np.float32)
    for c in range(NCH):
        gi = c * C + i
        cmask[:, c * C:(c + 1) * C] = np.where(
            i[:, None] < i[None, :], (gi + 1.0)[:, None], 0.0
        ).astype(np.float32)
        consts[:, c] = (gi + 1.0).astype(np.float32)
        consts[:, NCH + c] = (1.0 / (gi + 1.0)).astype(np.float32)
    return cmask, consts


def _build(big_dt=BIG_DT):
    """Build + compile the per-core Bass module (SPMD: same program, 8 cores)."""
    key = big_dt
    if key in _compiled:
        return _compiled[key]

    import concourse.bacc as bacc
    import concourse.bass as bass
    import concourse.mybir as mybir
    import concourse.tile as tile
    from concourse.masks import make_identity

    f32 = mybir.dt.float32
    i32 = mybir.dt.int32
    if big_dt == "f32r":
        mm_dt = mybir.dt.float32r
    elif big_dt == "f32":
        mm_dt = mybir.dt.float32
    elif big_dt == "bf16":
        mm_dt = mybir.dt.bfloat16
    else:
        raise ValueError(big_dt)
    mm_4byte = big_dt in ("f32r", "f32")

    nc = bacc.Bacc(
        "TRN2",
        target_bir_lowering=False,
        debug=False,
        enable_asserts=False,
        num_devices=NCORES,
    )

    bx_d = nc.dram_tensor("bx", [ROWS, H], mm_dt, kind="ExternalInput").ap()
    idx_d = nc.dram_tensor("idx", [C, BPC * NCH], i32, kind="ExternalInput").ap()
    # fused gather table: row v = [ae[v] | (ae @ w)[v]] in bf16 (A = E @ w
    # precomputed on host; one indirect DMA yields both E and A rows per token,
    # and bf16 rows keep the on-device transposes single-pass)
    eaw_d = nc.dram_tensor("eaw", [V, 2 * D], mybir.dt.bfloat16, kind="ExternalInput").ap()
    cm_d = nc.dram_tensor("cmask", [C, NCH * C], f32, kind="ExternalInput").ap()
    ct_d = nc.dram_tensor("consts", [C, 2 * NCH], f32, kind="ExternalInput").ap()
    out_d = nc.dram_tensor("out", [ROWS, H], f32, kind="ExternalOutput").ap()

    mult = mybir.AluOpType.mult
    add = mybir.AluOpType.add

    with tile.TileContext(nc) as tc:
        with (
            tc.tile_pool(name="const", bufs=1) as cpool,
            tc.tile_pool(name="bxp", bufs=6) as bxpool,
            tc.tile_pool(name="outp", bufs=4) as outpool,
            tc.tile_pool(name="sm", bufs=4) as smpool,
            tc.tile_pool(name="eap", bufs=6) as eapool,
            tc.tile_pool(name="mp", bufs=2) as mpool,
            tc.tile_pool(name="ps_et", bufs=1, space="PSUM") as ps_et,
            tc.tile_pool(name="ps_at", bufs=1, space="PSUM") as ps_at,
            tc.tile_pool(name="ps_s", bufs=2, space="PSUM") as ps_s,
            tc.tile_pool(name="ps_out", bufs=1, space="PSUM") as ps_out,
            tc.tile_pool(name="ps_m", bufs=1, space="PSUM") as ps_m,
        ):
            ident16 = cpool.tile([C, C], mybir.dt.bfloat16)
            make_identity(nc, ident16[:])
            # idx + consts first: every gather waits on idx_s, so it must not
            # queue behind the 1MB cmask on the sync DMA FIFO
            idx_s = cpool.tile([C, BPC * NCH], i32)
            nc.sync.dma_start(out=idx_s[:], in_=idx_d[:, :])
            consts_s = cpool.tile([C, 2 * NCH], f32)
            nc.sync.dma_start(out=consts_s[:], in_=ct_d[:, :])
            cmask_s = cpool.tile([C, NCH * C], f32)
            nc.sync.dma_start(out=cmask_s[:, 0:C], in_=cm_d[:, 0:C])
            nc.sync.dma_start(out=cmask_s[:, C:], in_=cm_d[:, C:])

            for b in range(BPC):
                M_p = ps_m.tile([D, H], f32, name=f"M_p_b{b}", tag="M_p")
                for c in range(NCH):
                    g = b * NCH + c
                    rows = slice(g * C, (g + 1) * C)

                    # one DMA loads two chunks' bx (fewer queue-issue slots,
                    # bigger transfers): [256, H] -> [128, 2H] side by side
                    if c % 2 == 0:
                        BX2 = bxpool.tile([C, 2 * H], mm_dt, name="BX2", tag="BX2")
                        nc.sync.dma_start(
                            out=BX2[:].rearrange("p (two h) -> p two h", two=2),
                            in_=bx_d[g * C:(g + 2) * C, :].rearrange(
                                "(two p) h -> p two h", two=2
                            ),
                        )
                    BX = BX2[:, :H] if c % 2 == 0 else BX2[:, H:]

                    if c > 0:
                        M_s = mpool.tile([D, H], mm_dt, name="M_s", tag="M_s")
                        nc.scalar.copy(out=M_s[:], in_=M_p[:])

                    EA = eapool.tile([C, 2 * D], mybir.dt.bfloat16, name="EA", tag="EA")
                    nc.gpsimd.indirect_dma_start(
                        out=EA[:],
                        out_offset=None,
                        in_=eaw_d[:, :],
                        in_offset=bass.IndirectOffsetOnAxis(
                            ap=idx_s[:, g:g + 1], axis=0
                        ),
                    )

                    et_p = ps_et.tile([D, C], mm_dt, name="et_p", tag="et_p")
                    at_p = ps_at.tile([D, C], mm_dt, name="at_p", tag="at_p")
                    et_v = et_p[:]
                    at_v = at_p[:]
                    nc.tensor.transpose(
                        out=et_v, in_=EA[:, 0:D], identity=ident16[:]
                    )
                    nc.tensor.transpose(
                        out=at_v, in_=EA[:, D:2 * D], identity=ident16[:]
                    )
                    Et = smpool.tile([D, C], mm_dt, name="Et", tag="Et")
                    nc.scalar.copy(out=Et[:], in_=et_v)
                    At = smpool.tile([D, C], mm_dt, name="At", tag="At")
                    nc.scalar.copy(out=At[:], in_=at_v)

                    # Ap = A * (i+1)  [C, D]   (row i = (i+1) a_i)
                    Ap = smpool.tile([C, D], mm_dt, name="Ap", tag="Ap")
                    nc.vector.tensor_scalar_mul(
                        out=Ap[:], in0=EA[:, D:2 * D], scalar1=consts_s[:, c:c + 1]
                    )

                    # S = At^T @ Et  [C, C];  St = S * cmask_c
                    s_p = ps_s.tile([C, C], f32, name="s_p", tag="s_p")
                    nc.tensor.matmul(
                        out=s_p[:], lhsT=At[:], rhs=Et[:], start=True, stop=True,
                    )
                    St = smpool.tile([C, C], mm_dt, name="St", tag="St")
                    nc.vector.tensor_tensor(
                        out=St[:],
                        in0=s_p[:],
                        in1=cmask_s[:, c * C:(c + 1) * C],
                        op=mult,
                    )

                    # M += Ap^T @ BX  [D, H]  (skip the never-read last update).
                    # skip_group_check: the sim's group guard can't express this
                    # read-between-accumulations pattern; the pending-zero
                    # accumulate semantics and Tile's HW sync are unaffected.
                    if c < NCH - 1:
                        for lo, hi in ((0, 512), (512, H)):
                            nc.tensor.matmul(
                                out=M_p[:, lo:hi],
                                lhsT=Ap[:],
                                rhs=BX[:, lo:hi],
                                start=(c == 0),
                                stop=True,
                                skip_group_check=True,
                            )

                    # acc = St^T @ BX (+ Et^T @ M)  [C, H]
                    out_p = ps_out.tile([C, H], f32, name="out_p", tag="out_p")
                    for lo, hi in ((0, 512), (512, H)):
                        nc.tensor.matmul(
                            out=out_p[:, lo:hi],
                            lhsT=St[:],
                            rhs=BX[:, lo:hi],
                            start=True,
                            stop=(c == 0),
                        )
                    if c > 0:
                        for lo, hi in ((0, 512), (512, H)):
                            nc.tensor.matmul(
                                out=out_p[:, lo:hi],
                                lhsT=Et[:],
                                rhs=M_s[:, lo:hi],
                                start=False,
                                stop=True,
                            )


                    # out = acc * (1/(j+1)) + bx
                    if c % 2 == 0:
                        OUT2 = outpool.tile([C, 2 * H], f32, name="OUT2", tag="OUT2")
                    out_s = OUT2[:, :H] if c % 2 == 0 else OUT2[:, H:]
                    nc.vector.scalar_tensor_tensor(
                        out=out_s,
                        in0=out_p[:],
                        scalar=consts_s[:, NCH + c:NCH + c + 1],
                        in1=BX[:, :].bitcast(f32) if mm_4byte else BX[:, :],
                        op0=mult,
                        op1=add,
                    )
                    if c % 2 == 1:
                        nc.sync.dma_start(
                            out=out_d[(g - 1) * C:(g + 1) * C, :].rearrange(
                                "(two p) h -> p two h", two=2
                            ),
                            in_=OUT2[:].rearrange("p (two h) -> p two h", two=2),
                        )

    # Adjacent PE matmuls sharing a stationary operand reload it redundantly;
    # mark the second of each such pair as pre-loaded (ldweights=True).
    for blk in nc.m.functions[0].blocks:
        last = None
        for inst in blk.instructions:
            if getattr(inst, "engine", None) != mybir.EngineType.PE:
                continue
            if not isinstance(inst, mybir.InstMatmult):
                if isinstance(inst, (mybir.InstLdweights,)):
                    last = None
                continue
            if (
                last is not None
                and not inst.is_transpose
                and not last.is_transpose
                and inst.ins[1].memref == last.ins[1].memref
                and inst.ins[1].offset == last.ins[1].offset
                and inst.ins[1].ap == last.ins[1].ap
            ):
                inst.ldweights = True
            last = inst

    nc.compile()
    _compiled[key] = nc
    return nc


def _in_maps(bert_x, x, ae, w, big_dt=BIG_DT):
    import ml_dtypes

    host_mm = np.float32 if big_dt in ("f32r", "f32") else ml_dtypes.bfloat16
    bert_x = np.ascontiguousarray(np.asarray(bert_x, dtype=np.float32).astype(host_mm))
    x = np.asarray(x)
    ae = np.asarray(ae, dtype=np.float32)
    w = np.asarray(w, dtype=np.float32)
    eaw = np.ascontiguousarray(
        np.concatenate([ae, ae @ w], axis=1).astype(ml_dtypes.bfloat16)
    )
    cmask, consts = _np_consts()
    # idx layout: [C, BPC*NCH] int32, column b*NCH+c = chunk c of local batch b
    xr = x.reshape(B, NCH, C).transpose(0, 2, 1).astype(np.int32)  # [B, C, NCH]
    maps = []
    for k in range(NCORES):
        maps.append(
            {
                "bx": bert_x[k * BPC:(k + 1) * BPC].reshape(ROWS, H),
                "idx": np.ascontiguousarray(
                    np.concatenate([xr[k * BPC + b] for b in range(BPC)], axis=1)
                ),
                "eaw": eaw,
                "cmask": cmask,
                "consts": consts,
            }
        )
    return maps


def _run(bert_x, x, ae, w, trace=False, big_dt=BIG_DT):
    from concourse import bass_utils

    nc = _build(big_dt)
    maps = _in_maps(bert_x, x, ae, w, big_dt)
    res = bass_utils.run_bass_kernel_spmd(
        nc, maps, core_ids=list(range(NCORES)), trace=trace
    )
    out = np.concatenate(
        [res.results[k]["out"].reshape(BPC, L, H) for k in range(NCORES)], axis=0
    )
    return out, res


def kernel(bert_x, x, ae, w):
    out, _ = _run(bert_x, x, ae, w, trace=False)
    return out
